# revision 1
# baseline (speedup 1.0000x reference)
"""NonLocalBlock2D (embedded-gaussian non-local attention) on 8 TRN2 NeuronCores.

Sharding: data-parallel over (batch, query-half). Core k handles sample b=k//2,
query rows h*3200:(h+1)*3200 with h=k%2. Attention keys/values are the full
6400 positions of that sample; the small 1x1-conv / BN params are replicated.

Per-core program (SPMD, one Bass module for all 8 cores):
  theta = Wth @ x_q + bth          [32,3200]  (stored 4x-replicated -> [128,3200])
  phi   = Wph @ x   + bph          [32,6400]  (4x-replicated -> [128,6400])
  gT    = x.T @ WgT + bg, chunked  [128,33] x 50  (col 32 = ones, for denominators)
  for each query block (512):
    for each key-chunk group (3 chunks of 128 keys, row-packed matmuls):
      fT = phi_chunk.T @ theta_blk -> PSUM [128,3x512]
      e  = exp(fT)                 -> SBUF  (ScalarE, the bottleneck engine)
      y  += gT_chunk.T @ e         -> PSUM [33,512] (row 32 accumulates denom)
    r = 1/denom; out = (WoT.T @ y) * r + x_residual   (BN folded into Wo/bias)

Host folds BN into the output conv, rotates x per-core so the query block is
always columns 0:3200 (softmax is invariant to key permutation), and stitches
the 8 [64,3200] results back into [4,64,80,80].
"""

import numpy as np

import concourse.bass as bass
import concourse.tile as tile
from concourse import bacc
from concourse import mybir
from concourse.bass import ts
from concourse.bass_utils import run_bass_kernel_spmd

B, C, HH, WW = 4, 64, 80, 80
N = HH * WW            # 6400 key positions per sample
NQ = N // 2            # 3200 query rows per core
INTER = 32
NCORES = 8

MC = 128               # keys per PE chunk
NMC = N // MC          # 50 chunks
PACK = 3               # chunks per packed f-matmul group (3 PSUM banks)
NB = 512               # query block size

F32 = mybir.dt.float32
F32R = mybir.dt.float32r
EXP = mybir.ActivationFunctionType.Exp
ADD = mybir.AluOpType.add
MULT = mybir.AluOpType.mult

BN_EPS = 1e-4

# r-broadcast strategy: 'dve' = stride-0 partition read on DVE,
# 'dma' = materialize via DMA partition-broadcast
RBC_MODE = 'dma'


def _blocks(total, size):
    off = 0
    while off < total:
        sz = min(size, total - off)
        yield off, sz
        off += sz


DEBUG = False


def _emit(tc, d, repeat=1):
    nc = tc.nc

    with tc.tile_pool(name="singles", bufs=1) as singles:
        wth = singles.tile([C, 128], F32, tag="wth")
        nc.sync.dma_start(wth[:], d["wth"][:])
        wph = singles.tile([C, 128], F32, tag="wph")
        nc.sync.dma_start(wph[:], d["wph"][:])
        wg = singles.tile([C, INTER], F32, tag="wg")
        nc.sync.dma_start(wg[:], d["wg"][:])
        wo = singles.tile([INTER, C], F32, tag="wo")
        nc.sync.dma_start(wo[:], d["wo"][:])
        bth = singles.tile([128, 1], F32, tag="bth")
        nc.sync.dma_start(bth[:], d["bth"][:])
        bph = singles.tile([128, 1], F32, tag="bph")
        nc.sync.dma_start(bph[:], d["bph"][:])
        bg = singles.tile([128, INTER], F32, tag="bg")
        nc.sync.dma_start(bg[:], d["bg"][0:1, :].partition_broadcast(128))
        ones64 = singles.tile([1, C], F32, tag="ones64")
        nc.vector.memset(ones64[:], 1.0)

        xfl = singles.tile([C, N], F32, tag="xf")
        for off, sz in _blocks(N, 3072):
            nc.sync.dma_start(xfl[:, off : off + sz], d["xf"][:, off : off + sz])
        xrl = singles.tile([C, NQ], F32, tag="xr")
        for off, sz in _blocks(NQ, 3072):
            nc.sync.dma_start(xrl[:, off : off + sz], d["xr"][:, off : off + sz])

        xfr = singles.tile([C, N], F32R, tag="xfr")
        nc.vector.tensor_copy(xfr[:], xfl[:])
        wthr = singles.tile([C, 128], F32R, tag="wthr")
        nc.vector.tensor_copy(wthr[:], wth[:])
        wphr = singles.tile([C, 128], F32R, tag="wphr")
        nc.vector.tensor_copy(wphr[:], wph[:])
        th = singles.tile([128, NQ], F32R, tag="th")

        ph = singles.tile([128, N], F32R, tag="ph")
        gt = singles.tile([128, NMC, INTER + 1], F32R, tag="gt")
        onescol = singles.tile([128, NMC], F32, tag="onescol")
        nc.vector.memset(onescol[:], 1.0)
        nc.vector.tensor_copy(gt[:, :, INTER : INTER + 1], onescol[:].rearrange("p (n o) -> p n o", o=1))

        # ---- input 1x1 convs ----
        for _rep in range(repeat):
            with tc.tile_pool(name="cpsum", bufs=4, space="PSUM") as cpsum:
                for off, sz in _blocks(NQ, NB):
                    pt = cpsum.tile([128, NB], F32, tag="cps")
                    nc.tensor.matmul(
                        pt[:, :sz],
                        lhsT=wthr[:],
                        rhs=xfr[:, off : off + sz],
                        start=True,
                        stop=True,
                    )
                    nc.vector.tensor_scalar_add(th[:, off : off + sz], pt[:, :sz], bth[:])
                for off, sz in _blocks(N, NB):
                    pp = cpsum.tile([128, NB], F32, tag="cps")
                    nc.tensor.matmul(
                        pp[:, :sz],
                        lhsT=wphr[:],
                        rhs=xfr[:, off : off + sz],
                        start=True,
                        stop=True,
                    )
                    nc.vector.tensor_scalar_add(ph[:, off : off + sz], pp[:, :sz], bph[:])
                for k in range(NMC):
                    pg = cpsum.tile([128, NB], F32, tag="cps")
                    nc.tensor.matmul(
                        pg[:, :INTER],
                        lhsT=xfl[:, ts(k, MC)],
                        rhs=wg[:],
                        start=True,
                        stop=True,
                    )
                    nc.vector.tensor_tensor(gt[:, k, :INTER], pg[:, :INTER], bg[:], op=ADD)

            if DEBUG:
                nc.sync.dma_start(d["d_th"][:], th[:].bitcast(F32))
                nc.sync.dma_start(d["d_ph"][:], ph[:].bitcast(F32))
                nc.sync.dma_start(d["d_gt"][:], gt[:].rearrange("p a b -> p (a b)").bitcast(F32))
                nc.sync.dma_start(d["d_bg"][:], bg[:])

            # ---- attention ----
            groups = []
            c0 = 0
            while c0 < NMC:
                gsz = min(PACK, NMC - c0)
                groups.append((c0, gsz))
                c0 += gsz

            att_blocks = [(0, 512), (512, 512), (1024, 512), (1536, 512), (2048, 512), (2560, 384), (2944, 256)]
            with tc.tile_pool(name="fpsum", bufs=2, space="PSUM") as fpsum, tc.tile_pool(
                name="ypsum", bufs=2, space="PSUM"
            ) as ypsum, tc.tile_pool(name="esb", bufs=3) as esb, tc.tile_pool(
                name="ep", bufs=2
            ) as ep:
                for n0, nb in att_blocks:
                    py = ypsum.tile([INTER + 1, NB], F32, tag="yz")
                    pending = [None]

                    def flush_y(py=py, nb=nb, pending=pending):
                        if pending[0] is None:
                            return
                        e, c0p, gszp = pending[0]
                        for j in range(gszp):
                            ch = c0p + j
                            nc.tensor.matmul(
                                py[:, :nb],
                                lhsT=gt[:, ch, :],
                                rhs=e[:, j, :nb],
                                start=(ch == 0),
                                stop=(ch == NMC - 1),
                            )
                        pending[0] = None

                    for c0g, gsz in groups:
                        pf = fpsum.tile([128, PACK, NB], F32, tag="f")
                        for j in range(gsz):
                            ch = c0g + j
                            bp = 32 * j
                            nc.tensor.matmul(
                                pf[:, j, :nb],
                                lhsT=ph[bp : bp + 32, ts(ch, MC)],
                                rhs=th[bp : bp + 32, n0 : n0 + nb],
                                start=True,
                                stop=True,
                            )
                        flush_y()
                        e = esb.tile([128, PACK, NB], F32R, tag="e")
                        nc.scalar.activation(e[:, :gsz, :nb], pf[:, :gsz, :nb], EXP)
                        if DEBUG and n0 == 0 and c0g == 0:
                            nc.sync.dma_start(d["d_e"][:], e[:].rearrange("p a b -> p (a b)").bitcast(F32))
                        pending[0] = (e, c0g, gsz)
                    flush_y()

                    # ---- block epilogue: normalize, output conv, residual ----
                    r = ep.tile([1, NB], F32, tag="r")
                    scr = ep.tile([1, NB], F32, tag="scr")
                    den = ep.tile([1, NB], F32, tag="den")
                    nc.vector.tensor_copy(den[:, :nb], py[INTER : INTER + 1, :nb])
                    nc.vector.reciprocal_approx_accurate(r[:, :nb], den[:, :nb], scr[:, :nb])
                    ysb = ep.tile([INTER, NB], F32, tag="ysb")
                    nc.vector.tensor_copy(ysb[:, :nb], py[:INTER, :nb])
                    if DEBUG and n0 == 0:
                        nc.sync.dma_start(d["d_ysb"][:], ysb[:, :nb])
                        nc.sync.dma_start(d["d_r"][:], r[:, :nb])
                    z = ypsum.tile([C, NB], F32, tag="yz")
                    nc.tensor.matmul(z[:, :nb], lhsT=wo[:], rhs=ysb[:, :nb], start=True, stop=True)
                    rbp = ypsum.tile([C, NB], F32, tag="yz")
                    nc.tensor.matmul(rbp[:, :nb], lhsT=ones64[:], rhs=r[:, :nb], start=True, stop=True)
                    rbc = ep.tile([C, NB], F32, tag="rbc")
                    nc.vector.tensor_copy(rbc[:, :nb], rbp[:, :nb])
                    if DEBUG and n0 == 0:
                        nc.sync.dma_start(d["d_rbc"][:], rbc[:, :nb])
                    t = ep.tile([C, NB], F32, tag="t")
                    nc.vector.tensor_tensor(t[:, :nb], z[:, :nb], rbc[:, :nb], op=MULT)
                    o = ep.tile([C, NB], F32, tag="o")
                    nc.vector.tensor_tensor(o[:, :nb], t[:, :nb], xrl[:, n0 : n0 + nb], op=ADD)
                    nc.sync.dma_start(d["out"][:, n0 : n0 + nb], o[:, :nb])


def build(repeat=1):
    nc = bacc.Bacc("TRN2", target_bir_lowering=False, debug=False)
    d = {}
    d["xf"] = nc.dram_tensor("xf", [C, N], F32, kind="ExternalInput").ap()
    d["xr"] = nc.dram_tensor("xr", [C, NQ], F32, kind="ExternalInput").ap()
    d["wth"] = nc.dram_tensor("wth", [C, 128], F32, kind="ExternalInput").ap()
    d["wph"] = nc.dram_tensor("wph", [C, 128], F32, kind="ExternalInput").ap()
    d["wg"] = nc.dram_tensor("wg", [C, INTER], F32, kind="ExternalInput").ap()
    d["wo"] = nc.dram_tensor("wo", [INTER, C], F32, kind="ExternalInput").ap()
    d["bth"] = nc.dram_tensor("bth", [128, 1], F32, kind="ExternalInput").ap()
    d["bph"] = nc.dram_tensor("bph", [128, 1], F32, kind="ExternalInput").ap()
    d["bg"] = nc.dram_tensor("bg", [1, INTER], F32, kind="ExternalInput").ap()
    d["out"] = nc.dram_tensor("out", [C, NQ], F32, kind="ExternalOutput").ap()
    if DEBUG:
        d["d_th"] = nc.dram_tensor("d_th", [128, NQ], F32, kind="ExternalOutput").ap()
        d["d_ph"] = nc.dram_tensor("d_ph", [128, N], F32, kind="ExternalOutput").ap()
        d["d_gt"] = nc.dram_tensor("d_gt", [128, NMC * (INTER + 1)], F32, kind="ExternalOutput").ap()
        d["d_bg"] = nc.dram_tensor("d_bg", [128, INTER], F32, kind="ExternalOutput").ap()
        d["d_e"] = nc.dram_tensor("d_e", [128, PACK * NB], F32, kind="ExternalOutput").ap()
        d["d_ysb"] = nc.dram_tensor("d_ysb", [INTER, NB], F32, kind="ExternalOutput").ap()
        d["d_r"] = nc.dram_tensor("d_r", [1, NB], F32, kind="ExternalOutput").ap()
        d["d_rbc"] = nc.dram_tensor("d_rbc", [C, NB], F32, kind="ExternalOutput").ap()
    with tile.TileContext(nc) as tc:
        _emit(tc, d, repeat=repeat)
    nc.compile()
    return nc


def make_in_maps(x, w_theta, b_theta, w_phi, b_phi, w_g, b_g,
                 w_out, b_out, bn_gamma, bn_beta, bn_mean, bn_var):
    x = np.ascontiguousarray(np.asarray(x, dtype=np.float32))
    w_theta = np.asarray(w_theta, np.float32)
    b_theta = np.asarray(b_theta, np.float32)
    w_phi = np.asarray(w_phi, np.float32)
    b_phi = np.asarray(b_phi, np.float32)
    w_g = np.asarray(w_g, np.float32)
    b_g = np.asarray(b_g, np.float32)
    w_out = np.asarray(w_out, np.float32)
    b_out = np.asarray(b_out, np.float32)
    bn_gamma = np.asarray(bn_gamma, np.float32)
    bn_beta = np.asarray(bn_beta, np.float32)
    bn_mean = np.asarray(bn_mean, np.float32)
    bn_var = np.asarray(bn_var, np.float32)

    inv = bn_gamma / np.sqrt(bn_var + BN_EPS)
    wo_folded = w_out * inv[:, None]                       # [64,32]
    bo_folded = (b_out - bn_mean) * inv + bn_beta          # [64]

    wth4 = np.ascontiguousarray(np.tile(w_theta.T, (1, 4)))   # [64,128]
    wph4 = np.ascontiguousarray(np.tile(w_phi.T, (1, 4)))     # [64,128]
    wg_r = np.ascontiguousarray(w_g.T)                        # [64,32]
    wo_l = np.ascontiguousarray(wo_folded.T)                  # [32,64]
    bth4 = np.ascontiguousarray(np.tile(b_theta, 4)[:, None])  # [128,1]
    bph4 = np.ascontiguousarray(np.tile(b_phi, 4)[:, None])    # [128,1]
    bg_r = np.ascontiguousarray(b_g[None, :])                  # [1,32]

    xflat = x.reshape(B, C, N)
    in_maps = []
    for core in range(NCORES):
        b, h = divmod(core, 2)
        xrot = np.ascontiguousarray(np.roll(xflat[b], -h * NQ, axis=1))
        xres = np.ascontiguousarray(xrot[:, :NQ] + bo_folded[:, None])
        in_maps.append(
            {
                "xf": xrot,
                "xr": xres,
                "wth": wth4,
                "wph": wph4,
                "wg": wg_r,
                "wo": wo_l,
                "bth": bth4,
                "bph": bph4,
                "bg": bg_r,
            }
        )
    return in_maps


def assemble_out(results):
    out = np.empty((B, C, N), np.float32)
    for core in range(NCORES):
        b, h = divmod(core, 2)
        out[b][:, h * NQ : (h + 1) * NQ] = results[core]["out"]
    return out.reshape(B, C, HH, WW)


_NC_CACHE = [None]


def kernel(**inputs):
    if _NC_CACHE[0] is None:
        _NC_CACHE[0] = build()
    nc = _NC_CACHE[0]
    in_maps = make_in_maps(**inputs)
    res = run_bass_kernel_spmd(nc, in_maps, core_ids=list(range(NCORES)))
    return assemble_out(res.results)



# revision 10
# speedup vs baseline: 1.5000x; 1.5000x over previous
"""NonLocalBlock2D (embedded-gaussian non-local attention) on 8 TRN2 NeuronCores.

Sharding: data-parallel over (batch, query-half). Core k handles sample b=k//2,
query rows h*3200:(h+1)*3200 with h=k%2. Attention keys/values are the full
6400 positions of that sample; the small 1x1-conv / BN params are replicated.

Per-core program (SPMD, one Bass module for all 8 cores):
  th = Wth @ x_q + bth            [32,3200] f32r (4x-replicated -> [128,3200])
  ph = Wph @ x   + bph            [32,6400] f32r (4x-replicated -> [128,6400])
  gz = x^T @ (Wg Wo_folded) + bgz [6400,65] bf16, chunked [128,65] x 50
       (col 64 == 1: accumulates softmax denominators; BN folded into Wo)
  for each query block (4x512 + 3x384) and each 128-key chunk:
    fT = ph_chunk.T @ th_blk            -> PSUM [128,<=512]   (TensorE)
    e  = exp(fT) -> bf16 SBUF, engine-rotated:
           ScalarE: true Exp activation
           DVE/GPSIMD: Schraudolph bit-trick bf16 exp in one tensor_scalar:
             bf16_bits(e) ~= int16(f * 128/ln2 + (127*128 + c))
    y[q,0:65] += e_slice.T @ gz_chunk   -> PSUM [128 queries, 65]  (TensorE)
         (e_slice [128 keys, 128 queries] is the stationary operand, so each
          chunk streams only 65 columns instead of 512 -> 4x fewer PE cycles)
  epilogue per block: r = 1/y[:,64]; out = y[:,0:64]*r + x_residual^T (fused
  scalar_tensor_tensor on DVE/GPSIMD); DMA out in query-major layout.

Host folds BN+output-conv into gz, rotates x per-core so the query block is
always columns 0:3200 (softmax is invariant to key permutation), transposes
residual/output to query-major, and stitches 8 results into [4,64,80,80].
"""

import numpy as np
import ml_dtypes

import concourse.bass as bass
import concourse.tile as tile
from concourse import bacc
from concourse import mybir
from concourse.bass import ts
from concourse.bass_utils import run_bass_kernel_spmd

B, C, HH, WW = 4, 64, 80, 80
N = HH * WW            # 6400 key positions per sample
NQ = N // 2            # 3200 query rows per core
NCORES = 8

MC = 128               # keys per PE chunk
NMC = N // MC          # 50 chunks
CO = 65                # gz columns: 64 output channels + 1 ones column
NS = NQ // 128         # 25 query slices of 128

F32 = mybir.dt.float32
F32R = mybir.dt.float32r
BF16 = mybir.dt.bfloat16
I16 = mybir.dt.int16
EXP = mybir.ActivationFunctionType.Exp
ADD = mybir.AluOpType.add
MULT = mybir.AluOpType.mult

BN_EPS = 1e-4

# query blocks: all >=256 cols (f32r full rate) and multiples of 128
ATT_BLOCKS = [(0, 512), (512, 512), (1024, 512), (1536, 512),
              (2048, 384), (2432, 384), (2816, 384)]

PACK = 2               # key chunks per f PSUM tile / exp instruction
LAG = 2                # f -> exp -> y software-pipeline depth (chunk pairs)
FBUFS = LAG + 1        # PSUM bufs for f tiles (3 x 2 banks) + 2 ypsum = 8 of 8

# exp engine rotation per chunk-pair: 0=ScalarE (exact), 1=DVE (Schraudolph)
# GPSIMD cannot read PSUM on TRN2, so it gets no exp work.
ROT = [0, 1, 0, 1, 0, 1, 0, 1, 0, 1, 0, 1, 0, 1, 0, 1, 0, 1, 0, 0]

SCH_A = 128.0 / float(np.log(2.0))
SCH_B = 127.0 * 128.0 - 4.25    # centered for truncating f32->i16 conversion

DEBUG = False


def _blocks(total, size):
    off = 0
    while off < total:
        sz = min(size, total - off)
        yield off, sz
        off += sz


def _emit(tc, d):
    nc = tc.nc

    with tc.tile_pool(name="singles", bufs=1) as singles:
        wth = singles.tile([C, 128], F32R, tag="wth")
        nc.sync.dma_start(wth[:], d["wth"][:])
        wph = singles.tile([C, 128], F32R, tag="wph")
        nc.sync.dma_start(wph[:], d["wph"][:])
        wgz = singles.tile([CO, CO], BF16, tag="wgz")
        nc.sync.dma_start(wgz[:], d["wgz"][:])
        bth = singles.tile([128, 1], F32, tag="bth")
        nc.sync.dma_start(bth[:], d["bth"][:])
        bph = singles.tile([128, 1], F32, tag="bph")
        nc.sync.dma_start(bph[:], d["bph"][:])
        xrt = singles.tile([128, NS * C], F32, tag="xrt")
        nc.sync.dma_start(xrt[:], d["xrt"][:])

        xf = singles.tile([C, N], F32R, tag="xf")
        xb = singles.tile([CO, N], BF16, tag="xb")
        for off, sz in _blocks(N, 3200):
            nc.sync.dma_start(xf[:, off : off + sz], d["xf"][:, off : off + sz])
            nc.sync.dma_start(xb[:, off : off + sz], d["xb"][:, off : off + sz])

        xfr = xf[:]
        wthr = wth[:]
        wphr = wph[:]

        th = singles.tile([128, NQ], F32R, tag="th")
        ph = singles.tile([128, N], F32R, tag="ph")
        gt = singles.tile([128, NMC, CO], BF16, tag="gt")

        def drain_add(i, dst, src, bias):
            if i % 2 == 0:
                nc.scalar.add(dst, src, bias)
            else:
                nc.vector.tensor_scalar_add(dst, src, bias)

        def drain_copy(i, dst, src):
            if i % 2 == 0:
                nc.scalar.copy(dst, src)
            else:
                nc.vector.tensor_copy(dst, src)

        # ---- input 1x1 convs ----
        di = 0
        with tc.tile_pool(name="cpsum", bufs=4, space="PSUM") as cpsum:
            for off, sz in _blocks(NQ, 512):
                pt = cpsum.tile([128, 512], F32, tag="cps")
                nc.tensor.matmul(pt[:, :sz], lhsT=wthr, rhs=xfr[:, off : off + sz],
                                 start=True, stop=True)
                drain_add(di, th[:, off : off + sz], pt[:, :sz], bth[:])
                di += 1
            for off, sz in _blocks(N, 512):
                pp = cpsum.tile([128, 512], F32, tag="cps")
                nc.tensor.matmul(pp[:, :sz], lhsT=wphr, rhs=xfr[:, off : off + sz],
                                 start=True, stop=True)
                drain_add(di, ph[:, off : off + sz], pp[:, :sz], bph[:])
                di += 1
            # gz conv: 4 key chunks share one PSUM bank as a single accum group
            for k0 in range(0, NMC, 4):
                kn = min(4, NMC - k0)
                pg = cpsum.tile([128, 4, CO], F32, tag="cpg")
                for j in range(kn):
                    nc.tensor.matmul(pg[:, j, :], lhsT=xb[:, ts(k0 + j, MC)],
                                     rhs=wgz[:], start=(j == 0), stop=(j == kn - 1))
                drain_copy(di, gt[:, k0 : k0 + kn, :], pg[:, :kn, :])
                di += 1

        if DEBUG:
            nc.sync.dma_start(d["d_th"][:], th[:].bitcast(F32))
            nc.sync.dma_start(d["d_ph"][:], ph[:].bitcast(F32))
            nc.sync.dma_start(d["d_gt"][:], gt[:].rearrange("p a b -> p (a b)"))

        # ---- attention ----
        with tc.tile_pool(name="fpsum", bufs=FBUFS, space="PSUM") as fpsum, \
             tc.tile_pool(name="ypsum", bufs=2, space="PSUM") as ypsum, \
             tc.tile_pool(name="esb", bufs=FBUFS) as esb, \
             tc.tile_pool(name="ep", bufs=2) as ep, \
             tc.tile_pool(name="ob", bufs=8) as ob:
            gi = 0
            for q0, nb in ATT_BLOCKS:
                nsl = nb // 128
                py = ypsum.tile([128, 4, CO], F32, tag="py")
                pend = []

                def flush_one(py=py, nsl=nsl, pend=pend):
                    e, c0, cn = pend.pop(0)
                    for j in range(cn):
                        ch = c0 + j
                        for s in range(nsl):
                            nc.tensor.matmul(
                                py[:, s, :],
                                lhsT=e[:, j, ts(s, 128)],
                                rhs=gt[:, ch, :],
                                start=(ch == 0 and s == 0),
                                stop=(ch == NMC - 1 and s == nsl - 1),
                            )

                for c0 in range(0, NMC, PACK):
                    cn = min(PACK, NMC - c0)
                    pf = fpsum.tile([128, PACK, 512], F32, tag="pf")
                    for j in range(cn):
                        ch = c0 + j
                        band = 32 * (ch % 3)
                        nc.tensor.matmul(pf[:, j, :nb],
                                         lhsT=ph[band : band + 32, ts(ch, MC)],
                                         rhs=th[band : band + 32, q0 : q0 + nb],
                                         start=True, stop=True)
                    if len(pend) >= LAG:
                        flush_one()
                    eng = ROT[gi % len(ROT)]
                    gi += 1
                    e = esb.tile([128, PACK, 512], BF16, tag="e")
                    if eng == 0:
                        nc.scalar.activation(e[:, :cn, :nb], pf[:, :cn, :nb], EXP)
                    else:
                        nc.vector.tensor_scalar(
                            e[:, :cn, :nb].bitcast(I16), pf[:, :cn, :nb],
                            SCH_A, SCH_B, op0=MULT, op1=ADD)
                    pend.append((e, c0, cn))
                while pend:
                    flush_one()

                # ---- block epilogue: normalize, add residual, store ----
                r = ep.tile([128, 4], F32, tag="r")
                rs = ep.tile([128, 4], F32, tag="rs")
                den = py[:, :nsl, CO - 1 : CO].rearrange("p a b -> p (a b)")
                nc.vector.reciprocal_approx_accurate(r[:, :nsl], den, rs[:, :nsl])
                for s in range(nsl):
                    S = q0 // 128 + s
                    t = ob.tile([128, C], F32, tag="t")
                    o = ob.tile([128, C], F32, tag="o")
                    nc.scalar.mul(t[:], py[:, s, 0:C], r[:, s : s + 1])
                    nc.gpsimd.tensor_tensor(o[:], t[:], xrt[:, S * C : (S + 1) * C],
                                            op=ADD)
                    nc.sync.dma_start(d["out"][:, S * C : (S + 1) * C], o[:])


def build():
    nc = bacc.Bacc("TRN2", target_bir_lowering=False, debug=False)
    d = {}
    d["xf"] = nc.dram_tensor("xf", [C, N], F32R, kind="ExternalInput").ap()
    d["xb"] = nc.dram_tensor("xb", [CO, N], BF16, kind="ExternalInput").ap()
    d["xrt"] = nc.dram_tensor("xrt", [128, NS * C], F32, kind="ExternalInput").ap()
    d["wth"] = nc.dram_tensor("wth", [C, 128], F32R, kind="ExternalInput").ap()
    d["wph"] = nc.dram_tensor("wph", [C, 128], F32R, kind="ExternalInput").ap()
    d["wgz"] = nc.dram_tensor("wgz", [CO, CO], BF16, kind="ExternalInput").ap()
    d["bth"] = nc.dram_tensor("bth", [128, 1], F32, kind="ExternalInput").ap()
    d["bph"] = nc.dram_tensor("bph", [128, 1], F32, kind="ExternalInput").ap()
    d["out"] = nc.dram_tensor("out", [128, NS * C], F32, kind="ExternalOutput").ap()
    if DEBUG:
        d["d_th"] = nc.dram_tensor("d_th", [128, NQ], F32, kind="ExternalOutput").ap()
        d["d_ph"] = nc.dram_tensor("d_ph", [128, N], F32, kind="ExternalOutput").ap()
        d["d_gt"] = nc.dram_tensor("d_gt", [128, NMC * CO], BF16, kind="ExternalOutput").ap()
        d["d_e"] = nc.dram_tensor("d_e", [128, 3 * 512], BF16, kind="ExternalOutput").ap()
    with tile.TileContext(nc) as tc:
        _emit(tc, d)
    nc.compile()
    return nc


def make_in_maps(x, w_theta, b_theta, w_phi, b_phi, w_g, b_g,
                 w_out, b_out, bn_gamma, bn_beta, bn_mean, bn_var):
    x = np.ascontiguousarray(np.asarray(x, dtype=np.float32))
    w_theta = np.asarray(w_theta, np.float32)
    b_theta = np.asarray(b_theta, np.float32)
    w_phi = np.asarray(w_phi, np.float32)
    b_phi = np.asarray(b_phi, np.float32)
    w_g = np.asarray(w_g, np.float32)
    b_g = np.asarray(b_g, np.float32)
    w_out = np.asarray(w_out, np.float32)
    b_out = np.asarray(b_out, np.float32)
    bn_gamma = np.asarray(bn_gamma, np.float32)
    bn_beta = np.asarray(bn_beta, np.float32)
    bn_mean = np.asarray(bn_mean, np.float32)
    bn_var = np.asarray(bn_var, np.float32)

    inv = bn_gamma / np.sqrt(bn_var + BN_EPS)
    wo_f = w_out * inv[:, None]                      # [64,32] folded output conv
    bo_f = (b_out - bn_mean) * inv + bn_beta         # [64]

    # gz conv: gz[key, j] = sum_c x[c,key] * Mgz[c,j] + bgz[j]; col 64 == 1
    Mgz = (wo_f @ w_g).T                             # [64(c_in), 64(j)]
    bgz = wo_f @ b_g                                 # [64]
    wgz = np.zeros((CO, CO), np.float32)
    wgz[:C, :C] = Mgz
    wgz[C, :C] = bgz
    wgz[C, C] = 1.0
    wgz_b = np.ascontiguousarray(wgz.astype(ml_dtypes.bfloat16))

    wth4 = np.ascontiguousarray(np.tile(w_theta.T, (1, 4)))    # [64,128]
    wph4 = np.ascontiguousarray(np.tile(w_phi.T, (1, 4)))      # [64,128]
    bth4 = np.ascontiguousarray(np.tile(b_theta, 4)[:, None])  # [128,1]
    bph4 = np.ascontiguousarray(np.tile(b_phi, 4)[:, None])    # [128,1]

    xflat = x.reshape(B, C, N)
    in_maps = []
    for core in range(NCORES):
        b, h = divmod(core, 2)
        xrot = np.ascontiguousarray(np.roll(xflat[b], -h * NQ, axis=1))
        xb = np.ones((CO, N), ml_dtypes.bfloat16)
        xb[:C] = xrot.astype(ml_dtypes.bfloat16)
        xres = xrot[:, :NQ] + bo_f[:, None]          # [64,3200]
        xrt = np.ascontiguousarray(
            xres.T.reshape(NS, 128, C).transpose(1, 0, 2).reshape(128, NS * C))
        in_maps.append(
            {
                "xf": xrot,
                "xb": np.ascontiguousarray(xb),
                "xrt": xrt,
                "wth": wth4,
                "wph": wph4,
                "wgz": wgz_b,
                "bth": bth4,
                "bph": bph4,
            }
        )
    return in_maps


def assemble_out(results):
    out = np.empty((B, C, N), np.float32)
    for core in range(NCORES):
        b, h = divmod(core, 2)
        r = np.asarray(results[core]["out"], np.float32)       # [128, 25*64]
        o3 = r.reshape(128, NS, C).transpose(1, 0, 2).reshape(NQ, C)
        out[b][:, h * NQ : (h + 1) * NQ] = o3.T
    return out.reshape(B, C, HH, WW)


_NC_CACHE = [None]


def kernel(**inputs):
    if _NC_CACHE[0] is None:
        _NC_CACHE[0] = build()
    nc = _NC_CACHE[0]
    in_maps = make_in_maps(**inputs)
    res = run_bass_kernel_spmd(nc, in_maps, core_ids=list(range(NCORES)))
    return assemble_out(res.results)


# revision 39
# speedup vs baseline: 1.6460x; 1.0973x over previous
"""NonLocalBlock2D (embedded-gaussian non-local attention) on 8 TRN2 NeuronCores.

Sharding: data-parallel over (batch, query-half). Core k handles sample b=k//2,
query rows h*3200:(h+1)*3200 with h=k%2. Attention keys/values are the full
6400 positions of that sample; the small 1x1-conv / BN params are replicated.

Per-core program (SPMD, one Bass module for all 8 cores):
  th = [Wth; bth] @ [x_q; 1]      [32,3200] f32r (4x-replicated -> [128,3200])
  ph = [Wph; bph] @ [x; 1]        [32,6400] f32r (4x-replicated -> [128,6400])
  gz = [x;1]^T @ [Wg Wo_fold; bg] [6400,65] bf16, chunked [128,65] x 50
       (col 64 == 1: accumulates softmax denominators; BN folded into Wo)
  for each query block (4x512 + 3x384) and each pair of 128-key chunks:
    fT = ph_chunk.T @ th_blk            -> PSUM [128,<=512] per chunk (TensorE)
    e  = exp(fT) -> bf16 SBUF, engine-rotated:
           ScalarE: true Exp activation
           DVE: Schraudolph bit-trick bf16 exp in one tensor_scalar:
             bf16_bits(e) ~= int16(f * 128/ln2 + (127*128 + c))
    y[q,0:65] += e_slice.T @ gz_chunk   -> PSUM [128 queries, 65]  (TensorE)
         (e_slice [128 keys, 128 queries] is the stationary operand, so each
          chunk streams only 65 columns instead of 512 -> 4x fewer PE cycles)
  epilogue per block: stage y to SBUF (ScalarE/DVE halves), then GPSIMD does
  normalize_recip (y[:,0:64]/y[:,64]) and the residual add; one DMA per block.

The input convs are interleaved into the first attention block's chunk-pair
stream (sharing the same PSUM pools) so TensorE never idles while DMA or the
SBUF drains catch up. The f->exp->y software pipeline runs with a flush lag of
3 chunk-pairs (PSUM: 3x2 f banks + 2 y banks = 8).

Host folds BN+output-conv into gz, rotates x per-core so the query block is
always columns 0:3200 (softmax is invariant to key permutation), transposes
residual/output to query-major, and stitches 8 results into [4,64,80,80].
"""

import numpy as np
import ml_dtypes

import concourse.bass as bass
import concourse.tile as tile
from concourse import bacc
from concourse import mybir
from concourse.bass import ts
from concourse.bass_utils import run_bass_kernel_spmd

B, C, HH, WW = 4, 64, 80, 80
N = HH * WW            # 6400 key positions per sample
NQ = N // 2            # 3200 query rows per core
NCORES = 8

MC = 128               # keys per PE chunk
NMC = N // MC          # 50 chunks
CO = 65                # gz columns: 64 output channels + 1 ones column
NS = NQ // 128         # 25 query slices of 128

F32 = mybir.dt.float32
F32R = mybir.dt.float32r
BF16 = mybir.dt.bfloat16
I16 = mybir.dt.int16
EXP = mybir.ActivationFunctionType.Exp
ADD = mybir.AluOpType.add
MULT = mybir.AluOpType.mult

BN_EPS = 1e-4

# query blocks: all >=256 cols (f32r full rate) and multiples of 128
ATT_BLOCKS = [(0, 512), (512, 512), (1024, 512), (1536, 512),
              (2048, 384), (2432, 384), (2816, 384)]

PACK = 2               # key chunks per f PSUM tile / exp instruction
LAG = 3                # f -> exp -> y flush lag (chunk pairs); e pool is LAG+1
FBUFS = 3              # PSUM bufs for f tiles (3 x 2 banks) + 2 ypsum = 8 of 8
EBUFS = LAG + 1        # SBUF e tiles so the deeper flush lag has live inputs

# exp engine rotation per chunk-pair: 0=ScalarE (exact), 1=DVE (Schraudolph)
# GPSIMD cannot read PSUM on TRN2, so it handles the SBUF-side epilogue.
ROT = [0, 1, 0, 1, 0, 1, 0, 1, 0, 1, 0, 1, 0]

CONV_BLOCKS_XM = [(0, 512, 512), (1024, 512, 512), (2048, 512, 512),
                  (3072, 512, 512), (4096, 512, 512), (5120, 512, 512), (6144, 256, 0)]

# conv units interleaved into attention block 0, one per chunk-pair, ordered so
# every unit lands before the attention stream consumes its output
INTERLEAVE = [("xm", 1), ("gz", 2), ("gz", 3), ("xm", 2), ("gz", 4), ("gz", 5),
              ("xm", 3), ("gz", 6), ("gz", 7), ("xm", 4), ("gz", 8), ("gz", 9),
              ("xm", 5), ("gz", 10), ("gz", 11), ("xm", 6), ("gz", 12)]

SCH_A = 128.0 / float(np.log(2.0))
SCH_B = 127.0 * 128.0 - 4.25    # centered for truncating f32->i16 conversion

DEBUG = False


def _blocks(total, size):
    off = 0
    while off < total:
        sz = min(size, total - off)
        yield off, sz
        off += sz


def _emit(tc, d):
    nc = tc.nc

    with tc.tile_pool(name="singles", bufs=1) as singles:
        # DMA order tuned for the first conv/attention pairs: weights and the
        # leading x columns first, residual last
        wxm = singles.tile([CO, CO], F32R, tag="wxm")
        nc.sync.dma_start(wxm[:], d["wxm"][:])
        xf = singles.tile([CO, N], F32R, tag="xf")
        nc.sync.dma_start(xf[:, 0:1024], d["xf"][:, 0:1024])
        wgz = singles.tile([CO, CO], BF16, tag="wgz")
        nc.sync.dma_start(wgz[:], d["wgz"][:])
        xb = singles.tile([CO, N], BF16, tag="xb")
        nc.sync.dma_start(xb[:, 0:1024], d["xb"][:, 0:1024])
        nc.sync.dma_start(xf[:, 1024:2560], d["xf"][:, 1024:2560])
        nc.sync.dma_start(xb[:, 1024:3200], d["xb"][:, 1024:3200])
        nc.sync.dma_start(xf[:, 2560:4480], d["xf"][:, 2560:4480])
        nc.sync.dma_start(xb[:, 3200:6400], d["xb"][:, 3200:6400])
        nc.sync.dma_start(xf[:, 4480:6400], d["xf"][:, 4480:6400])
        xrt = singles.tile([128, NS * C], F32, tag="xrt")
        nc.sync.dma_start(xrt[:], d["xrt"][:])

        # xm = (Wth^T Wph applied to x, biases folded): f = xf_q^T @ xm
        xm = singles.tile([CO, N], F32R, tag="xm")
        gt = singles.tile([128, NMC, CO], BF16, tag="gt")

        def drain_copy2(dst0, src0, dst1, src1):
            # split a drain across ScalarE and DVE so the SBUF copy of a conv
            # result is ready in half the time (the pipeline is latency-bound)
            nc.scalar.copy(dst0, src0)
            nc.vector.tensor_copy(dst1, src1)

        with tc.tile_pool(name="fpsum", bufs=FBUFS, space="PSUM") as fpsum, \
             tc.tile_pool(name="ypsum", bufs=2, space="PSUM") as ypsum, \
             tc.tile_pool(name="esb", bufs=EBUFS) as esb, \
             tc.tile_pool(name="ep", bufs=2) as ep, \
             tc.tile_pool(name="ob", bufs=2) as ob:

            def emit_conv_pair(dst, w, off, sz0, sz1):
                pt = fpsum.tile([128, PACK, 512], F32, tag="pf")
                nc.tensor.matmul(pt[:CO, 0, :sz0], lhsT=w,
                                 rhs=xf[:, off : off + sz0],
                                 start=True, stop=True)
                if sz1:
                    assert sz1 == sz0
                    nc.tensor.matmul(pt[:CO, 1, :sz1], lhsT=w,
                                     rhs=xf[:, off + sz0 : off + sz0 + sz1],
                                     start=True, stop=True)
                    drain_copy2(dst[:, off : off + sz0], pt[:CO, 0, :sz0],
                                dst[:, off + sz0 : off + 2 * sz0],
                                pt[:CO, 1, :sz0])
                else:
                    h = sz0 // 2
                    drain_copy2(dst[:, off : off + h], pt[:CO, 0, :h],
                                dst[:, off + h : off + sz0], pt[:CO, 0, h:sz0])

            def emit_gz(k0):
                kn = min(4, NMC - k0)
                pg4 = fpsum.tile([128, PACK, 512], F32, tag="pf")
                pg = pg4[:, 0, : 4 * CO].rearrange("p (a b) -> p a b", a=4)
                for j in range(kn):
                    nc.tensor.matmul(pg[:, j, :], lhsT=xb[:, ts(k0 + j, MC)],
                                     rhs=wgz[:], start=(j == 0), stop=(j == kn - 1))
                hk = max(kn // 2, 1)
                drain_copy2(gt[:, k0 : k0 + hk, :], pg[:, :hk, :],
                            gt[:, k0 + hk : k0 + kn, :], pg[:, hk:kn, :])

            def emit_unit(u):
                kind, idx = u
                if kind == "xm":
                    emit_conv_pair(xm, wxm[:], *CONV_BLOCKS_XM[idx])
                else:
                    emit_gz(idx * 4)

            # PE warmup: dummy matmuls on a memset tile while the x DMAs land,
            # so the tensor engine is at max p-state when real work arrives
            wu = singles.tile([128, 512], BF16, tag="wu")
            nc.gpsimd.memset(wu[:], 1.0)
            for _ in range(6):
                pw = fpsum.tile([128, PACK, 512], F32, tag="pf")
                nc.tensor.matmul(pw[:, 0, :], lhsT=wu[0:65, 0:128],
                                 rhs=wu[0:65, :], start=True, stop=True)

            # lead-in: everything attention pairs 0..3 need
            emit_conv_pair(xm, wxm[:], *CONV_BLOCKS_XM[0])
            emit_gz(0)
            emit_gz(4)

            if DEBUG:
                nc.sync.dma_start(d["d_xm"][:], xm[:].bitcast(F32))
                nc.sync.dma_start(d["d_gt"][:], gt[:].rearrange("p a b -> p (a b)"))

            ui = 0
            gi = 0
            last_q0 = ATT_BLOCKS[-1][0]
            for q0, nb in ATT_BLOCKS:
                nsl = nb // 128
                py = ypsum.tile([128, 4, CO], F32, tag="py")
                pend = []

                def flush_one(py=py, nsl=nsl, pend=pend):
                    e, c0, cn = pend.pop(0)
                    for j in range(cn):
                        ch = c0 + j
                        for s in range(nsl):
                            nc.tensor.matmul(
                                py[:, s, :],
                                lhsT=e[:, j, ts(s, 128)],
                                rhs=gt[:, ch, :],
                                start=(ch == 0 and s == 0),
                                stop=(ch == NMC - 1 and s == nsl - 1),
                            )

                for c0 in range(0, NMC, PACK):
                    cn = min(PACK, NMC - c0)
                    pf = fpsum.tile([128, PACK, 512], F32, tag="pf")
                    for j in range(cn):
                        ch = c0 + j
                        nc.tensor.matmul(pf[:, j, :nb],
                                         lhsT=xm[:, ts(ch, MC)],
                                         rhs=xf[:, q0 : q0 + nb],
                                         start=True, stop=True)
                    if len(pend) >= LAG:
                        flush_one()
                    if ui < len(INTERLEAVE):
                        emit_unit(INTERLEAVE[ui])
                        ui += 1
                    eng = ROT[gi % len(ROT)]
                    gi += 1
                    e = esb.tile([128, PACK, 512], BF16, tag="e")
                    tail = q0 == last_q0 and c0 >= NMC - 6 * PACK and cn == 2
                    if tail:
                        # end of kernel: split the pair across both engines so
                        # the final exp->y drain is latency-minimal
                        nc.scalar.activation(e[:, 0, :nb], pf[:, 0, :nb], EXP)
                        nc.vector.tensor_scalar(
                            e[:, 1, :nb].bitcast(I16), pf[:, 1, :nb],
                            SCH_A, SCH_B, op0=MULT, op1=ADD)
                    elif eng == 0:
                        nc.scalar.activation(e[:, :cn, :nb], pf[:, :cn, :nb], EXP)
                    else:
                        nc.vector.tensor_scalar(
                            e[:, :cn, :nb].bitcast(I16), pf[:, :cn, :nb],
                            SCH_A, SCH_B, op0=MULT, op1=ADD)
                    pend.append((e, c0, cn))
                while pend:
                    flush_one()

                # ---- block epilogue: stage y to SBUF (ScalarE/DVE halves),
                # then GPSIMD normalizes and adds the residual; one DMA/block
                sy = ep.tile([128, 4, CO], F32, tag="sy")
                h = max(nsl // 2, 1)
                nc.scalar.copy(sy[:, :h, :], py[:, :h, :])
                if nsl > h:
                    nc.vector.tensor_copy(sy[:, h:nsl, :], py[:, h:nsl, :])
                ot = ob.tile([128, 4, C], F32, tag="ot")
                for s in range(nsl):
                    S = q0 // 128 + s
                    tn = ob.tile([128, C], F32, tag="tn", bufs=4)
                    nc.gpsimd.normalize_recip(tn[:], sy[:, s, 0:C],
                                              sy[:, s, CO - 1 : CO])
                    nc.gpsimd.tensor_tensor(ot[:, s, :], tn[:],
                                            xrt[:, S * C : (S + 1) * C], op=ADD)
                S0 = q0 // 128
                nc.sync.dma_start(
                    d["out"][:, S0 * C : (S0 + nsl) * C],
                    ot[:, :nsl, :].rearrange("p a b -> p (a b)"))


def build():
    nc = bacc.Bacc("TRN2", target_bir_lowering=False, debug=False)
    d = {}
    d["xf"] = nc.dram_tensor("xf", [CO, N], F32R, kind="ExternalInput").ap()
    d["xb"] = nc.dram_tensor("xb", [CO, N], BF16, kind="ExternalInput").ap()
    d["xrt"] = nc.dram_tensor("xrt", [128, NS * C], F32, kind="ExternalInput").ap()
    d["wxm"] = nc.dram_tensor("wxm", [CO, CO], F32R, kind="ExternalInput").ap()
    d["wgz"] = nc.dram_tensor("wgz", [CO, CO], BF16, kind="ExternalInput").ap()
    d["out"] = nc.dram_tensor("out", [128, NS * C], F32, kind="ExternalOutput").ap()
    if DEBUG:
        d["d_xm"] = nc.dram_tensor("d_xm", [CO, N], F32, kind="ExternalOutput").ap()
        d["d_gt"] = nc.dram_tensor("d_gt", [128, NMC * CO], BF16, kind="ExternalOutput").ap()
    with tile.TileContext(nc) as tc:
        _emit(tc, d)
    nc.compile()
    return nc


def make_in_maps(x, w_theta, b_theta, w_phi, b_phi, w_g, b_g,
                 w_out, b_out, bn_gamma, bn_beta, bn_mean, bn_var):
    x = np.ascontiguousarray(np.asarray(x, dtype=np.float32))
    w_theta = np.asarray(w_theta, np.float32)
    b_theta = np.asarray(b_theta, np.float32)
    w_phi = np.asarray(w_phi, np.float32)
    b_phi = np.asarray(b_phi, np.float32)
    w_g = np.asarray(w_g, np.float32)
    b_g = np.asarray(b_g, np.float32)
    w_out = np.asarray(w_out, np.float32)
    b_out = np.asarray(b_out, np.float32)
    bn_gamma = np.asarray(bn_gamma, np.float32)
    bn_beta = np.asarray(bn_beta, np.float32)
    bn_mean = np.asarray(bn_mean, np.float32)
    bn_var = np.asarray(bn_var, np.float32)

    inv = bn_gamma / np.sqrt(bn_var + BN_EPS)
    wo_f = w_out * inv[:, None]                      # [64,32] folded output conv
    bo_f = (b_out - bn_mean) * inv + bn_beta         # [64]

    # gz conv: gz[key, j] = sum_c x[c,key] * Mgz[c,j] + bgz[j]; col 64 == 1
    Mgz = (wo_f @ w_g).T                             # [64(c_in), 64(j)]
    bgz = wo_f @ b_g                                 # [64]
    wgz = np.zeros((CO, CO), np.float32)
    wgz[:C, :C] = Mgz
    wgz[C, :C] = bgz
    wgz[C, C] = 1.0
    wgz_b = np.ascontiguousarray(wgz.astype(ml_dtypes.bfloat16))

    # fused theta/phi conv: f[q,k] = xf_ext[:,q] . xm_ext[:,k] with
    # xm_ext = wxm^T @ x_ext.  wxm[r, c] rows r = x channels + ones row,
    # cols c = xm channels (64 fused channels + the bth^T phi column).
    wxm = np.empty((CO, CO), np.float32)
    wxm[:C, :C] = w_phi.T @ w_theta          # (Wth^T Wph)^T
    wxm[C, :C] = w_theta.T @ b_phi           # bias: theta^T bph term
    wxm[:C, C] = w_phi.T @ b_theta           # bth^T Wph x term
    wxm[C, C] = float(b_theta @ b_phi)       # bth . bph
    wxm = np.ascontiguousarray(wxm)

    xflat = x.reshape(B, C, N)
    in_maps = []
    for core in range(NCORES):
        b, h = divmod(core, 2)
        xrot = np.ascontiguousarray(np.roll(xflat[b], -h * NQ, axis=1))
        xe = np.ones((CO, N), np.float32)
        xe[:C] = xrot
        xb = np.ones((CO, N), ml_dtypes.bfloat16)
        xb[:C] = xrot.astype(ml_dtypes.bfloat16)
        xres = xrot[:, :NQ] + bo_f[:, None]          # [64,3200]
        xrt = np.ascontiguousarray(
            xres.T.reshape(NS, 128, C).transpose(1, 0, 2).reshape(128, NS * C))
        in_maps.append(
            {
                "xf": np.ascontiguousarray(xe),
                "xb": np.ascontiguousarray(xb),
                "xrt": xrt,
                "wxm": wxm,
                "wgz": wgz_b,
            }
        )
    return in_maps


def assemble_out(results):
    out = np.empty((B, C, N), np.float32)
    for core in range(NCORES):
        b, h = divmod(core, 2)
        r = np.asarray(results[core]["out"], np.float32)       # [128, 25*64]
        o3 = r.reshape(128, NS, C).transpose(1, 0, 2).reshape(NQ, C)
        out[b][:, h * NQ : (h + 1) * NQ] = o3.T
    return out.reshape(B, C, HH, WW)


_NC_CACHE = [None]


def kernel(**inputs):
    if _NC_CACHE[0] is None:
        _NC_CACHE[0] = build()
    nc = _NC_CACHE[0]
    in_maps = make_in_maps(**inputs)
    res = run_bass_kernel_spmd(nc, in_maps, core_ids=list(range(NCORES)))
    return assemble_out(res.results)


# revision 54
# speedup vs baseline: 1.7253x; 1.0482x over previous
"""NonLocalBlock2D (embedded-gaussian non-local attention) on 8 TRN2 NeuronCores.

Sharding: data-parallel over (batch, query-half). Core k handles sample b=k//2,
query rows h*3200:(h+1)*3200 with h=k%2. Attention keys/values are the full
6400 positions of that sample; the small 1x1-conv / BN params are replicated.

Per-core program (SPMD, one Bass module for all 8 cores):
  th = [Wth; bth] @ [x_q; 1]      [32,3200] f32r (4x-replicated -> [128,3200])
  ph = [Wph; bph] @ [x; 1]        [32,6400] f32r (4x-replicated -> [128,6400])
  gz = [x;1]^T @ [Wg Wo_fold; bg] [6400,65] bf16, chunked [128,65] x 50
       (col 64 == 1: accumulates softmax denominators; BN folded into Wo)
  for each query block (4x512 + 3x384) and each pair of 128-key chunks:
    fT = ph_chunk.T @ th_blk            -> PSUM [128,<=512] per chunk (TensorE)
    e  = exp(fT) -> bf16 SBUF, engine-rotated:
           ScalarE: true Exp activation
           DVE: Schraudolph bit-trick bf16 exp in one tensor_scalar:
             bf16_bits(e) ~= int16(f * 128/ln2 + (127*128 + c))
    y[q,0:65] += e_slice.T @ gz_chunk   -> PSUM [128 queries, 65]  (TensorE)
         (e_slice [128 keys, 128 queries] is the stationary operand, so each
          chunk streams only 65 columns instead of 512 -> 4x fewer PE cycles)
  epilogue per block: stage y to SBUF (ScalarE/DVE halves), then GPSIMD does
  normalize_recip (y[:,0:64]/y[:,64]) and the residual add; one DMA per block.

The input convs are interleaved into the first attention block's chunk-pair
stream (sharing the same PSUM pools) so TensorE never idles while DMA or the
SBUF drains catch up. The f->exp->y software pipeline runs with a flush lag of
3 chunk-pairs (PSUM: 3x2 f banks + 2 y banks = 8).

Host folds BN+output-conv into gz, rotates x per-core so the query block is
always columns 0:3200 (softmax is invariant to key permutation), transposes
residual/output to query-major, and stitches 8 results into [4,64,80,80].
"""

import numpy as np
import ml_dtypes

import concourse.bass as bass
import concourse.tile as tile
from concourse import bacc
from concourse import mybir
from concourse.bass import ts
from concourse.bass_utils import run_bass_kernel_spmd

B, C, HH, WW = 4, 64, 80, 80
N = HH * WW            # 6400 key positions per sample
NQ = N // 2            # 3200 query rows per core
NCORES = 8

MC = 128               # keys per PE chunk
NMC = N // MC          # 50 chunks
CO = 65                # gz columns: 64 output channels + 1 ones column
NS = NQ // 128         # 25 query slices of 128

F32 = mybir.dt.float32
F32R = mybir.dt.float32r
BF16 = mybir.dt.bfloat16
I16 = mybir.dt.int16
EXP = mybir.ActivationFunctionType.Exp
ADD = mybir.AluOpType.add
MULT = mybir.AluOpType.mult

BN_EPS = 1e-4

# query blocks: all >=256 cols (f32r full rate) and multiples of 128
ATT_BLOCKS = [(0, 512), (512, 512), (1024, 512), (1536, 512),
              (2048, 384), (2432, 384), (2816, 384)]

PACK = 2               # key chunks per f PSUM tile / exp instruction
LAG = 4                # f -> exp -> y flush lag (chunk pairs); e pool is LAG+1
FBUFS = 3              # PSUM bufs for f tiles (3 x 2 banks) + 2 ypsum = 8 of 8
EBUFS = LAG + 1        # SBUF e tiles so the deeper flush lag has live inputs

# exp engine rotation per chunk-pair: 0=ScalarE (exact), 1=DVE (Schraudolph)
# GPSIMD cannot read PSUM on TRN2, so it handles the SBUF-side epilogue.
ROT = [0, 1, 0, 1, 0, 1, 0, 1, 0, 1, 0, 1, 0]

CONV_BLOCKS_XM = [(0, 512, 512), (1024, 512, 512), (2048, 512, 512),
                  (3072, 512, 512), (4096, 512, 512), (5120, 512, 512), (6144, 256, 0)]

# conv units interleaved into attention block 0, one per chunk-pair, ordered so
# every unit lands before the attention stream consumes its output
INTERLEAVE = [("xm", 1), ("gz", 2), ("gz", 3), ("xm", 2), ("gz", 4), ("gz", 5),
              ("xm", 3), ("gz", 6), ("gz", 7), ("xm", 4), ("gz", 8), ("gz", 9),
              ("xm", 5), ("gz", 10), ("gz", 11), ("xm", 6), ("gz", 12)]

SCH_A = 128.0 / float(np.log(2.0))
SCH_B = 127.0 * 128.0 - 4.25    # centered for truncating f32->i16 conversion

DEBUG = False


def _blocks(total, size):
    off = 0
    while off < total:
        sz = min(size, total - off)
        yield off, sz
        off += sz


def _emit(tc, d):
    nc = tc.nc

    with tc.tile_pool(name="singles", bufs=1) as singles:
        # DMA order tuned for the first conv/attention pairs: weights and the
        # leading x columns first, residual last
        wxm = singles.tile([CO, CO], F32R, tag="wxm")
        nc.sync.dma_start(wxm[:], d["wxm"][:])
        xf = singles.tile([CO, N], F32R, tag="xf")
        nc.sync.dma_start(xf[:, 0:512], d["xf"][:, 0:512])
        wgz = singles.tile([CO, CO], BF16, tag="wgz")
        nc.sync.dma_start(wgz[:], d["wgz"][:])
        nc.sync.dma_start(xf[:, 512:1024], d["xf"][:, 512:1024])
        nc.sync.dma_start(xf[:, 1024:2560], d["xf"][:, 1024:2560])
        nc.sync.dma_start(xf[:, 2560:4480], d["xf"][:, 2560:4480])
        nc.sync.dma_start(xf[:, 4480:6400], d["xf"][:, 4480:6400])
        xrt = singles.tile([128, NS * C], F32, tag="xrt")
        nc.sync.dma_start(xrt[:], d["xrt"][:])

        # bf16 x for the gz conv is derived on GPSIMD (SBUF->SBUF) instead of
        # DMA'd: Pool is otherwise idle and this shortens the input DMA stream
        xb = singles.tile([CO, N], BF16, tag="xb")

        # xm = (Wth^T Wph applied to x, biases folded): f = xf_q^T @ xm
        xm = singles.tile([CO, N], F32R, tag="xm")
        gt = singles.tile([128, NMC, CO], BF16, tag="gt")

        def drain_copy(eng, dst, src):
            if eng == 0:
                nc.scalar.copy(dst, src)
            else:
                nc.vector.tensor_copy(dst, src)

        with tc.tile_pool(name="fpsum", bufs=FBUFS, space="PSUM") as fpsum, \
             tc.tile_pool(name="ypsum", bufs=2, space="PSUM") as ypsum, \
             tc.tile_pool(name="esb", bufs=EBUFS) as esb, \
             tc.tile_pool(name="ep", bufs=2) as ep, \
             tc.tile_pool(name="ob", bufs=2) as ob:

            def emit_conv_pair(dst, w, off, sz0, sz1, eng=None):
                pt = fpsum.tile([128, PACK, 512], F32, tag="pf")
                nc.tensor.matmul(pt[:CO, 0, :sz0], lhsT=w,
                                 rhs=xf[:, off : off + sz0],
                                 start=True, stop=True)
                if sz1:
                    assert sz1 == sz0
                    nc.tensor.matmul(pt[:CO, 1, :sz1], lhsT=w,
                                     rhs=xf[:, off + sz0 : off + sz0 + sz1],
                                     start=True, stop=True)
                if eng is None:
                    # lead-in: split across both engines for minimum latency
                    if sz1:
                        drain_copy(0, dst[:, off : off + sz0], pt[:CO, 0, :sz0])
                        drain_copy(1, dst[:, off + sz0 : off + 2 * sz0],
                                   pt[:CO, 1, :sz0])
                    else:
                        h = sz0 // 2
                        drain_copy(0, dst[:, off : off + h], pt[:CO, 0, :h])
                        drain_copy(1, dst[:, off + h : off + sz0],
                                   pt[:CO, 0, h:sz0])
                elif sz1:
                    drain_copy(eng,
                               dst[:, off : off + 2 * sz0]
                               .rearrange("p (a b) -> p a b", a=2),
                               pt[:CO, :2, :sz0])
                else:
                    drain_copy(eng, dst[:, off : off + sz0], pt[:CO, 0, :sz0])

            def emit_gz(k0, eng=None):
                kn = min(4, NMC - k0)
                off = k0 * MC
                nc.gpsimd.tensor_copy(xb[:, off : off + kn * MC],
                                      xf[:, off : off + kn * MC].bitcast(F32))
                pg4 = fpsum.tile([128, PACK, 512], F32, tag="pf")
                pg = pg4[:, 0, : 4 * CO].rearrange("p (a b) -> p a b", a=4)
                for j in range(kn):
                    nc.tensor.matmul(pg[:, j, :], lhsT=xb[:, ts(k0 + j, MC)],
                                     rhs=wgz[:], start=(j == 0), stop=(j == kn - 1))
                if eng is None:
                    hk = max(kn // 2, 1)
                    drain_copy(0, gt[:, k0 : k0 + hk, :], pg[:, :hk, :])
                    drain_copy(1, gt[:, k0 + hk : k0 + kn, :], pg[:, hk:kn, :])
                else:
                    drain_copy(eng, gt[:, k0 : k0 + kn, :], pg[:, :kn, :])

            def emit_unit(u, eng):
                kind, idx = u
                if kind == "xm":
                    emit_conv_pair(xm, wxm[:], *CONV_BLOCKS_XM[idx], eng=eng)
                else:
                    emit_gz(idx * 4, eng=eng)

            # PE warmup: dummy matmuls on a memset tile while the x DMAs land,
            # so the tensor engine is at max p-state when real work arrives
            wu = singles.tile([128, 512], BF16, tag="wu")
            nc.vector.memset(wu[:], 1.0)
            for _ in range(6):
                pw = fpsum.tile([128, PACK, 512], F32, tag="pf")
                nc.tensor.matmul(pw[:, 0, :], lhsT=wu[0:65, 0:128],
                                 rhs=wu[0:65, :], start=True, stop=True)

            # lead-in: everything attention pairs 0..3 need
            emit_conv_pair(xm, wxm[:], *CONV_BLOCKS_XM[0])
            emit_gz(0)
            emit_gz(4)

            if DEBUG:
                nc.sync.dma_start(d["d_xm"][:], xm[:].bitcast(F32))
                nc.sync.dma_start(d["d_gt"][:], gt[:].rearrange("p a b -> p (a b)"))

            ui = 0
            gi = 0
            last_q0 = ATT_BLOCKS[-1][0]
            def emit_epilogue(py, nsl, q0, last):
                # stage y to SBUF (ScalarE/DVE halves), then normalize + add
                # residual.  Steady state: GPSIMD does it (keeps the exp
                # engines free).  Final block: DVE, which is idle by then and
                # has no Q7 launch latency; per-slice DMA to drain fast.
                S0 = q0 // 128
                if last:
                    # latency-minimal final epilogue: DVE reads PSUM directly,
                    # one DMA for the whole block
                    r = ep.tile([128, 4], F32, tag="r")
                    rs = ep.tile([128, 4], F32, tag="rs")
                    den = py[:, :nsl, CO - 1 : CO].rearrange("p a b -> p (a b)")
                    nc.vector.reciprocal_approx_accurate(r[:, :nsl], den,
                                                         rs[:, :nsl])
                    ol = ob.tile([128, 4, C], F32, tag="ot")
                    for s in range(nsl):
                        S = S0 + s
                        nc.vector.scalar_tensor_tensor(
                            ol[:, s, :], py[:, s, 0:C], r[:, s : s + 1],
                            xrt[:, S * C : (S + 1) * C], op0=MULT, op1=ADD)
                    nc.sync.dma_start(
                        d["out"][:, S0 * C : (S0 + nsl) * C],
                        ol[:, :nsl, :].rearrange("p a b -> p (a b)"))
                    return
                sy = ep.tile([128, 4, CO], F32, tag="sy")
                h = max(nsl // 2, 1)
                nc.scalar.copy(sy[:, :h, :], py[:, :h, :])
                if nsl > h:
                    nc.vector.tensor_copy(sy[:, h:nsl, :], py[:, h:nsl, :])
                ot = ob.tile([128, 4, C], F32, tag="ot")
                for s in range(nsl):
                    S = S0 + s
                    tn = ob.tile([128, C], F32, tag="tn", bufs=4)
                    nc.gpsimd.normalize_recip(tn[:], sy[:, s, 0:C],
                                              sy[:, s, CO - 1 : CO])
                    nc.gpsimd.tensor_tensor(ot[:, s, :], tn[:],
                                            xrt[:, S * C : (S + 1) * C], op=ADD)
                nc.sync.dma_start(
                    d["out"][:, S0 * C : (S0 + nsl) * C],
                    ot[:, :nsl, :].rearrange("p a b -> p (a b)"))

            pend = []

            def flush_one():
                e, c0, cn, py, nsl, q0 = pend.pop(0)
                for j in range(cn):
                    ch = c0 + j
                    for s in range(nsl):
                        nc.tensor.matmul(
                            py[:, s, :],
                            lhsT=e[:, j, ts(s, 128)],
                            rhs=gt[:, ch, :],
                            start=(ch == 0 and s == 0),
                            stop=(ch == NMC - 1 and s == nsl - 1),
                        )
                if c0 + cn >= NMC:
                    emit_epilogue(py, nsl, q0, last=(q0 == last_q0))

            for q0, nb in ATT_BLOCKS:
                nsl = nb // 128
                py = ypsum.tile([128, 4, CO], F32, tag="py")
                for c0 in range(0, NMC, PACK):
                    cn = min(PACK, NMC - c0)
                    pf = fpsum.tile([128, PACK, 512], F32, tag="pf")
                    for j in range(cn):
                        ch = c0 + j
                        nc.tensor.matmul(pf[:, j, :nb],
                                         lhsT=xm[:, ts(ch, MC)],
                                         rhs=xf[:, q0 : q0 + nb],
                                         start=True, stop=True)
                    if len(pend) >= LAG:
                        flush_one()
                    if ui < len(INTERLEAVE):
                        # drain on the engine not running the next pair's exp
                        emit_unit(INTERLEAVE[ui], 1 - ROT[(gi + 1) % len(ROT)])
                        ui += 1
                    eng = ROT[gi % len(ROT)]
                    gi += 1
                    e = esb.tile([128, PACK, 512], BF16, tag="e")
                    tail = q0 == last_q0 and c0 >= NMC - 6 * PACK and cn == 2
                    if tail:
                        # end of kernel: split the pair across both engines so
                        # the final exp->y drain is latency-minimal
                        nc.scalar.activation(e[:, 0, :nb], pf[:, 0, :nb], EXP)
                        nc.vector.tensor_scalar(
                            e[:, 1, :nb].bitcast(I16), pf[:, 1, :nb],
                            SCH_A, SCH_B, op0=MULT, op1=ADD)
                    elif eng == 0:
                        nc.scalar.activation(e[:, :cn, :nb], pf[:, :cn, :nb], EXP)
                    else:
                        nc.vector.tensor_scalar(
                            e[:, :cn, :nb].bitcast(I16), pf[:, :cn, :nb],
                            SCH_A, SCH_B, op0=MULT, op1=ADD)
                    pend.append((e, c0, cn, py, nsl, q0))
            while pend:
                flush_one()


def build():
    nc = bacc.Bacc("TRN2", target_bir_lowering=False, debug=False)
    d = {}
    d["xf"] = nc.dram_tensor("xf", [CO, N], F32R, kind="ExternalInput").ap()
    d["xrt"] = nc.dram_tensor("xrt", [128, NS * C], F32, kind="ExternalInput").ap()
    d["wxm"] = nc.dram_tensor("wxm", [CO, CO], F32R, kind="ExternalInput").ap()
    d["wgz"] = nc.dram_tensor("wgz", [CO, CO], BF16, kind="ExternalInput").ap()
    d["out"] = nc.dram_tensor("out", [128, NS * C], F32, kind="ExternalOutput").ap()
    if DEBUG:
        d["d_xm"] = nc.dram_tensor("d_xm", [CO, N], F32, kind="ExternalOutput").ap()
        d["d_gt"] = nc.dram_tensor("d_gt", [128, NMC * CO], BF16, kind="ExternalOutput").ap()
    with tile.TileContext(nc) as tc:
        _emit(tc, d)
    nc.compile()
    return nc


def make_in_maps(x, w_theta, b_theta, w_phi, b_phi, w_g, b_g,
                 w_out, b_out, bn_gamma, bn_beta, bn_mean, bn_var):
    x = np.ascontiguousarray(np.asarray(x, dtype=np.float32))
    w_theta = np.asarray(w_theta, np.float32)
    b_theta = np.asarray(b_theta, np.float32)
    w_phi = np.asarray(w_phi, np.float32)
    b_phi = np.asarray(b_phi, np.float32)
    w_g = np.asarray(w_g, np.float32)
    b_g = np.asarray(b_g, np.float32)
    w_out = np.asarray(w_out, np.float32)
    b_out = np.asarray(b_out, np.float32)
    bn_gamma = np.asarray(bn_gamma, np.float32)
    bn_beta = np.asarray(bn_beta, np.float32)
    bn_mean = np.asarray(bn_mean, np.float32)
    bn_var = np.asarray(bn_var, np.float32)

    inv = bn_gamma / np.sqrt(bn_var + BN_EPS)
    wo_f = w_out * inv[:, None]                      # [64,32] folded output conv
    bo_f = (b_out - bn_mean) * inv + bn_beta         # [64]

    # gz conv: gz[key, j] = sum_c x[c,key] * Mgz[c,j] + bgz[j]; col 64 == 1
    Mgz = (wo_f @ w_g).T                             # [64(c_in), 64(j)]
    bgz = wo_f @ b_g                                 # [64]
    wgz = np.zeros((CO, CO), np.float32)
    wgz[:C, :C] = Mgz
    wgz[C, :C] = bgz
    wgz[C, C] = 1.0
    wgz_b = np.ascontiguousarray(wgz.astype(ml_dtypes.bfloat16))

    # fused theta/phi conv: f[q,k] = xf_ext[:,q] . xm_ext[:,k] with
    # xm_ext = wxm^T @ x_ext.  wxm[r, c] rows r = x channels + ones row,
    # cols c = xm channels (64 fused channels + the bth^T phi column).
    wxm = np.empty((CO, CO), np.float32)
    wxm[:C, :C] = w_phi.T @ w_theta          # (Wth^T Wph)^T
    wxm[C, :C] = w_theta.T @ b_phi           # bias: theta^T bph term
    wxm[:C, C] = w_phi.T @ b_theta           # bth^T Wph x term
    wxm[C, C] = float(b_theta @ b_phi)       # bth . bph
    wxm = np.ascontiguousarray(wxm)

    xflat = x.reshape(B, C, N)
    in_maps = []
    for core in range(NCORES):
        b, h = divmod(core, 2)
        xrot = np.ascontiguousarray(np.roll(xflat[b], -h * NQ, axis=1))
        xe = np.ones((CO, N), np.float32)
        xe[:C] = xrot
        xres = xrot[:, :NQ] + bo_f[:, None]          # [64,3200]
        xrt = np.ascontiguousarray(
            xres.T.reshape(NS, 128, C).transpose(1, 0, 2).reshape(128, NS * C))
        in_maps.append(
            {
                "xf": np.ascontiguousarray(xe),
                "xrt": xrt,
                "wxm": wxm,
                "wgz": wgz_b,
            }
        )
    return in_maps


def assemble_out(results):
    out = np.empty((B, C, N), np.float32)
    for core in range(NCORES):
        b, h = divmod(core, 2)
        r = np.asarray(results[core]["out"], np.float32)       # [128, 25*64]
        o3 = r.reshape(128, NS, C).transpose(1, 0, 2).reshape(NQ, C)
        out[b][:, h * NQ : (h + 1) * NQ] = o3.T
    return out.reshape(B, C, HH, WW)


_NC_CACHE = [None]


def kernel(**inputs):
    if _NC_CACHE[0] is None:
        _NC_CACHE[0] = build()
    nc = _NC_CACHE[0]
    in_maps = make_in_maps(**inputs)
    res = run_bass_kernel_spmd(nc, in_maps, core_ids=list(range(NCORES)))
    return assemble_out(res.results)
